# revision 5
# baseline (speedup 1.0000x reference)
"""Trainium2 Bass kernel for nn_DensityLoss (retrieval kNN hinge loss).

Computes mean(relu(topk_smallest_dist(x_pred, x_target, k) - 1.0)).

Strategy (8 NeuronCores, SPMD, x_pred rows sharded; 1024 rows/core):
  Every one of the 16384 dot products per row must be drained out of PSUM
  by exactly one compute-engine read (DMA and Pool cannot touch PSUM), so
  the drain is split between ScalarE and the DVE using wide (2048-elem)
  instructions to amortize fixed overheads:

  - Host sorts targets by ||b||^2 and permutes them into regions:
      V (positions     0.. 7167): DVE tensor_reduce(axis=X) over a
        [128, c, 16] PSUM view drains AND fully folds in one op ->
        448 chunk maxima of width 16.
      W (positions  7168.. 8191 and 12288..16383): ScalarE evac shipped
        raw fp16 -> 5120 width-1 near-exact scores handled on host.
      F (positions  8192..12287): two 2048-wide ScalarE evac slabs,
        pair-merged by one DVE fp16 2x tensor_max -> width-2 chunks.
  - TensorE computes 2*a.b in bf16 (fp32 PSUM), 512-col matmuls.
  - Host: chunk scores = chunk-max(2ab) - min b2 per chunk (consecutive
    sorted ranks inside each chunk keep the bound tight). Top-C chunks
    per family are rescored exactly in float64 -> top-k -> hinge -> mean.
"""

import numpy as np

N_CORES = 8
N_PRED = 8192
N_TGT = 16384
DIM = 128
ROWS_PER_CORE = N_PRED // N_CORES  # 1024
ROWTILES = ROWS_PER_CORE // 128    # 8
BANK = 512                         # fp32 PSUM bank, matmul max N
SG = 2048                          # super-group width (4 banks)
WIN = 16                           # V-chunk width
V_END = 7168                       # V region positions [0, 7168)
NVCH = V_END // WIN                # 448 V-chunks
NW = 5120                          # width-1 raw columns
CV = 12                            # top chunks, V family (width 16)
CF = 12                            # top chunks, F family (width 2)
KW = 16                            # raw width-1 candidates
HINGE = 1.0

_CACHE = {}


def _build_nc():
    import concourse.bacc as bacc
    import concourse.bass as bass
    import concourse.mybir as mybir
    import concourse.tile as tile

    dt = mybir.dt
    nc = bacc.Bacc(
        "TRN2",
        target_bir_lowering=False,
        debug=False,
        num_devices=N_CORES,
    )
    a_t = nc.dram_tensor("a_t", [DIM, ROWS_PER_CORE], dt.bfloat16, kind="ExternalInput")
    b_t = nc.dram_tensor("b_t", [DIM, N_TGT], dt.bfloat16, kind="ExternalInput")
    vout = nc.dram_tensor(
        "vout", [ROWTILES, 128, NVCH], dt.float16, kind="ExternalOutput"
    )
    fab = nc.dram_tensor(
        "fab", [ROWTILES, 128, SG], dt.float16, kind="ExternalOutput"
    )
    wraw = nc.dram_tensor(
        "wraw", [ROWTILES, 128, NW], dt.float16, kind="ExternalOutput"
    )

    AX = mybir.AxisListType.X
    MAX = mybir.AluOpType.max

    # Per-rowtile super-group schedule (SG index = position // 2048),
    # interleaved so ScalarE and DVE drain concurrently:
    #   SG0-2: V windowed-reduce (DVE). SG3: first half V, second half
    #   raw evac (ScalarE). SG4,5: slabs s0,s1 -> F01 merge. SG6,7:
    #   slabs s2,s3 shipped raw.
    SEQ = [0, 4, 1, 5, 2, 6, 3, 7]

    with tile.TileContext(nc) as tc:
        with (
            tc.tile_pool(name="const", bufs=1) as cpool,
            tc.tile_pool(name="psum", bufs=2, space="PSUM") as ppool,
            tc.tile_pool(name="slab", bufs=2) as spool,
            tc.tile_pool(name="vo", bufs=2) as vpool,
            tc.tile_pool(name="fo", bufs=2) as fpool,
        ):
            bt_sb = cpool.tile([DIM, N_TGT], dt.bfloat16)
            at_sb = cpool.tile([DIM, ROWS_PER_CORE], dt.bfloat16)

            nc.sync.dma_start(out=at_sb[:], in_=a_t[:])
            # load b_t slices in first-use order
            for g in SEQ:
                for j in range(SG // BANK):
                    s = g * (SG // BANK) + j
                    sl = bass.ts(s, BANK)
                    nc.sync.dma_start(out=bt_sb[:, sl], in_=b_t[:, sl])

            for rt in range(ROWTILES):
                lhsT = at_sb[:, bass.ts(rt, 128)]
                # slab: h(1024) | s0 | s1 | s2 | s3  (fp16)
                slab = spool.tile([128, 1024 + 4 * SG], dt.float16, tag="slab")
                vtile = vpool.tile([128, NVCH], dt.float16, tag="vt")
                ftile = fpool.tile([128, SG], dt.float16, tag="fo")

                def sl_s(i):  # slab slice for S-slab i (0..3)
                    return slab[:, 1024 + i * SG : 1024 + (i + 1) * SG]

                for g in SEQ:
                    ps = ppool.tile([128, SG], dt.float32)
                    for j in range(SG // BANK):
                        c = g * (SG // BANK) + j
                        nc.tensor.matmul(
                            ps[:, bass.ts(j, BANK)],
                            lhsT,
                            bt_sb[:, bass.ts(c, BANK)],
                            start=True,
                            stop=True,
                        )
                    if g < 3:  # V super-group: windowed drain+fold
                        nc.vector.tensor_reduce(
                            vtile[:, g * 128 : (g + 1) * 128],
                            ps[:].rearrange("p (c w) -> p c w", w=WIN),
                            axis=AX,
                            op=MAX,
                        )
                    elif g == 3:  # half V, half raw evac
                        nc.vector.tensor_reduce(
                            vtile[:, 384:448],
                            ps[:, 0:1024].rearrange("p (c w) -> p c w", w=WIN),
                            axis=AX,
                            op=MAX,
                        )
                        nc.scalar.copy(slab[:, 0:1024], ps[:, 1024:2048])
                    else:  # S super-group: ScalarE evac
                        i = g - 4
                        nc.scalar.copy(sl_s(i), ps[:])
                        if i == 1:
                            nc.vector.tensor_max(
                                ftile[:], sl_s(0), sl_s(1)
                            )
                nc.sync.dma_start(out=vout[rt][:], in_=vtile[:])
                nc.sync.dma_start(out=fab[rt][:], in_=ftile[:])
                # wraw: h | s2 | s3
                nc.sync.dma_start(out=wraw[rt][:, 0:1024], in_=slab[:, 0:1024])
                nc.sync.dma_start(
                    out=wraw[rt][:, 1024 : 1024 + SG], in_=sl_s(2)
                )
                nc.sync.dma_start(
                    out=wraw[rt][:, 1024 + SG : NW], in_=sl_s(3)
                )

    nc.compile()
    return nc


def _get_nc():
    if "nc" not in _CACHE:
        _CACHE["nc"] = _build_nc()
    return _CACHE["nc"]


def _prep(x_pred, x_target):
    """Host-side layout: sort targets by b2, permute into V/W/F regions."""
    import ml_dtypes

    b2 = np.einsum(
        "ij,ij->i", x_target.astype(np.float64), x_target.astype(np.float64)
    )
    order = np.argsort(b2, kind="stable")
    perm = np.empty(N_TGT, np.int64)
    # positions [0, 8192): rank == position (V region + first raw block)
    perm[:8192] = order[:8192]
    # F region: slab s in {0,1} at positions 8192 + 2048*s + j.
    # F01 chunk j = {slab0 col j, slab1 col j} <- ranks 8192 + 2j + s
    j_idx = np.arange(SG)
    for s in range(2):
        perm[8192 + SG * s + j_idx] = order[8192 + 2 * j_idx + s]
    # positions [12288, 16384): rank == position (raw blocks s2, s3)
    perm[12288:] = order[12288:]

    a_t = np.ascontiguousarray(2.0 * x_pred.T).astype(ml_dtypes.bfloat16)
    b_t = np.ascontiguousarray(x_target[perm].T).astype(ml_dtypes.bfloat16)
    return a_t, b_t, b2, order, perm


def _host_finish(x_pred, x_target, vo, fb, wr, b2, order, perm, k):
    n = x_pred.shape[0]
    a64 = x_pred.astype(np.float64)
    b64 = x_target.astype(np.float64)
    a2 = np.einsum("ij,ij->i", a64, a64)

    # V family: 448 width-16 chunks (consecutive ranks; first = min b2)
    b2min_v = b2[order[np.arange(NVCH) * WIN]].astype(np.float32)
    chv = np.argpartition(-(vo - b2min_v[None, :]), CV, axis=1)[:, :CV]
    tid_v = order[:V_END].reshape(NVCH, WIN)[chv].reshape(n, CV * WIN)

    # F01 family: 2048 width-2 chunks
    b2min_f = b2[order[8192 + 2 * np.arange(SG)]].astype(np.float32)
    chf = np.argpartition(-(fb - b2min_f[None, :]), CF, axis=1)[:, :CF]
    tid_f = (
        order[8192 : 8192 + 2 * SG].reshape(SG, 2)[chf].reshape(n, CF * 2)
    )

    # W: width-1 near-exact (2ab in fp16); d2 = a2 + b2 - 2ab
    # wraw cols: [0:1024] positions 7168..8191; [1024:5120] pos 12288..16383
    w_ids = np.concatenate([order[V_END:8192], perm[12288:]])
    d2_w = (
        a2[:, None].astype(np.float32)
        + b2[w_ids].astype(np.float32)[None, :]
        - wr
    )
    chw = np.argpartition(d2_w, KW, axis=1)[:, :KW]
    tid_w = w_ids[chw]

    tids = np.concatenate([tid_v, tid_f, tid_w], axis=1)
    vals = np.empty((n, k))
    B = 2048
    for s in range(0, n, B):
        t = tids[s : s + B]
        dots = np.einsum("rd,rcd->rc", a64[s : s + B], b64[t], optimize=True)
        d2 = a2[s : s + B, None] + b2[t] - 2.0 * dots
        vals[s : s + B] = np.partition(d2, k - 1, axis=1)[:, :k]
    d = np.sqrt(np.maximum(vals, 0.0))
    return np.float32(np.maximum(d - HINGE, 0.0).mean(dtype=np.float64))


def _host_exact(x_pred, x_target, k):
    """Exact fallback (never expected in practice)."""
    a = x_pred.astype(np.float32)
    b = x_target.astype(np.float32)
    a2 = np.sum(a * a, axis=1)[:, None]
    b2 = np.sum(b * b, axis=1)[None, :]
    out = np.empty((a.shape[0], k), np.float64)
    B = 1024
    for s in range(0, a.shape[0], B):
        d2 = a2[s : s + B] + b2 - 2.0 * (a[s : s + B] @ b.T)
        out[s : s + B] = np.partition(d2, k - 1, axis=1)[:, :k].astype(np.float64)
    d = np.sqrt(np.maximum(out, 0.0))
    return np.float32(np.maximum(d - HINGE, 0.0).mean(dtype=np.float64))


def kernel(x_pred, x_target, top_k=5, _want_results=False):
    from concourse.bass_utils import run_bass_kernel_spmd

    x_pred = np.asarray(x_pred, dtype=np.float32)
    x_target = np.asarray(x_target, dtype=np.float32)
    k = int(top_k)
    if (
        k > CF * 2
        or x_pred.shape != (N_PRED, DIM)
        or x_target.shape != (N_TGT, DIM)
        or not np.isfinite(x_pred).all()
        or not np.isfinite(x_target).all()
        or float(np.abs(x_pred).max()) * float(np.abs(x_target).max()) * DIM
        > 2.0e4
    ):
        return _host_exact(x_pred, x_target, k)

    nc = _get_nc()
    a_t_full, b_t, b2, order, perm = _prep(x_pred, x_target)

    in_maps = []
    for c in range(N_CORES):
        in_maps.append(
            {
                "a_t": np.ascontiguousarray(
                    a_t_full[:, c * ROWS_PER_CORE : (c + 1) * ROWS_PER_CORE]
                ),
                "b_t": b_t,
            }
        )

    res = run_bass_kernel_spmd(nc, in_maps, list(range(N_CORES)))
    vo = np.concatenate(
        [
            res.results[c]["vout"].reshape(ROWS_PER_CORE, NVCH)
            for c in range(N_CORES)
        ],
        axis=0,
    ).astype(np.float32)
    fb = np.concatenate(
        [
            res.results[c]["fab"].reshape(ROWS_PER_CORE, SG)
            for c in range(N_CORES)
        ],
        axis=0,
    ).astype(np.float32)
    wr = np.concatenate(
        [
            res.results[c]["wraw"].reshape(ROWS_PER_CORE, NW)
            for c in range(N_CORES)
        ],
        axis=0,
    ).astype(np.float32)
    out = _host_finish(x_pred, x_target, vo, fb, wr, b2, order, perm, k)
    if _want_results:
        return out, res
    return out


# revision 6
# speedup vs baseline: 1.5061x; 1.5061x over previous
"""Trainium2 Bass kernel for nn_DensityLoss (retrieval kNN hinge loss).

Computes mean(relu(topk_smallest_dist(x_pred, x_target, k) - 1.0)).

Strategy (8 NeuronCores, SPMD, x_pred rows sharded; 1024 rows/core):
  Every one of the 16384 dot products per row must be drained out of PSUM
  by exactly one compute-engine read (DMA and Pool cannot touch PSUM).
  The drain is split between the DVE and ScalarE, each with its own
  2-deep PSUM tile rotation so both stream independently:

  - V region (positions    0.. 8191): DVE tensor_reduce(axis=X) over a
    [128, 64, 16] PSUM view drains AND folds a 1024-group in one op ->
    512 width-16 chunk maxima per row.
  - W region (positions 8192..16383): ScalarE evacuates fp32->fp16 and
    the slab is shipped raw -> 8192 width-1 near-exact scores per row.

  The kernel runs group-PAIR-major (outer: 8 (V,W) group pairs; inner:
  8 rowtiles), so compute starts as soon as the first b_t slices land
  instead of waiting for the whole 4 MiB load, and the TensorE stays
  continuously busy (p-state).

  Host: targets are b2-sorted (pure sort, no permutation needed: chunk =
  16 consecutive ranks). Chunk score = chunk-max(2ab) - min b2; top-CV
  V-chunks + top-KW W-columns are rescored exactly in float64 -> top-k
  -> hinge -> mean.
"""

import numpy as np

N_CORES = 8
N_PRED = 8192
N_TGT = 16384
DIM = 128
ROWS_PER_CORE = N_PRED // N_CORES  # 1024
ROWTILES = ROWS_PER_CORE // 128    # 8
BANK = 512                         # fp32 PSUM bank, matmul max N
GRP = 1024                         # drain group width (2 PSUM banks)
NPAIR = 8                          # (V, W) group pairs per rowtile
WIN = 16                           # V-chunk width
V_END = 8192                       # V region positions [0, 8192)
NVCH = V_END // WIN                # 512 V-chunks
NW = 8192                          # width-1 raw columns
CV = 12                            # top chunks, V family (width 16)
KW = 16                            # raw width-1 candidates
HINGE = 1.0

_CACHE = {}


def _build_nc():
    import concourse.bacc as bacc
    import concourse.bass as bass
    import concourse.mybir as mybir
    import concourse.tile as tile

    dt = mybir.dt
    nc = bacc.Bacc(
        "TRN2",
        target_bir_lowering=False,
        debug=False,
        num_devices=N_CORES,
    )
    a_t = nc.dram_tensor("a_t", [DIM, ROWS_PER_CORE], dt.bfloat16, kind="ExternalInput")
    b_t = nc.dram_tensor("b_t", [DIM, N_TGT], dt.bfloat16, kind="ExternalInput")
    vout = nc.dram_tensor(
        "vout", [ROWTILES, 128, NVCH], dt.float16, kind="ExternalOutput"
    )
    wraw = nc.dram_tensor(
        "wraw", [ROWTILES, 128, NW], dt.float16, kind="ExternalOutput"
    )

    AX = mybir.AxisListType.X
    MAX = mybir.AluOpType.max

    with tile.TileContext(nc) as tc:
        with (
            tc.tile_pool(name="const", bufs=1) as cpool,
            tc.tile_pool(name="vp", bufs=2, space="PSUM") as vpp,
            tc.tile_pool(name="sp", bufs=2, space="PSUM") as spp,
            tc.tile_pool(name="slab", bufs=4) as spool,
        ):
            bt_sb = cpool.tile([DIM, N_TGT], dt.bfloat16)
            at_sb = cpool.tile([DIM, ROWS_PER_CORE], dt.bfloat16)
            # per-rowtile V-chunk accumulators, DMA'd out at the end
            vt_sb = cpool.tile([128, ROWTILES, NVCH], dt.float16)

            nc.sync.dma_start(out=at_sb[:], in_=a_t[:])
            # b_t slices in first-use order: pair p uses cols
            # [1024p, 1024p+1024) and [8192+1024p, ...)
            for p in range(NPAIR):
                for half in (0, 1):
                    base = half * V_END + p * GRP
                    for j in range(GRP // BANK):
                        sl = bass.ts(base // BANK + j, BANK)
                        nc.sync.dma_start(out=bt_sb[:, sl], in_=b_t[:, sl])

            for p in range(NPAIR):
                vcols = p * GRP            # V-group column base
                wcols = V_END + p * GRP    # W-group column base
                for rt in range(ROWTILES):
                    lhsT = at_sb[:, bass.ts(rt, 128)]
                    pv = vpp.tile([128, GRP], dt.float32)
                    pw = spp.tile([128, GRP], dt.float32)
                    for j in range(GRP // BANK):
                        nc.tensor.matmul(
                            pv[:, bass.ts(j, BANK)],
                            lhsT,
                            bt_sb[:, bass.ts(vcols // BANK + j, BANK)],
                            start=True,
                            stop=True,
                        )
                    for j in range(GRP // BANK):
                        nc.tensor.matmul(
                            pw[:, bass.ts(j, BANK)],
                            lhsT,
                            bt_sb[:, bass.ts(wcols // BANK + j, BANK)],
                            start=True,
                            stop=True,
                        )
                    nc.vector.tensor_reduce(
                        vt_sb[:, rt, p * (GRP // WIN) : (p + 1) * (GRP // WIN)],
                        pv[:].rearrange("p (c w) -> p c w", w=WIN),
                        axis=AX,
                        op=MAX,
                    )
                    slab = spool.tile([128, GRP], dt.float16)
                    nc.scalar.copy(slab[:], pw[:])
                    nc.sync.dma_start(
                        out=wraw[rt][:, bass.ts(p, GRP)], in_=slab[:]
                    )
            for rt in range(ROWTILES):
                nc.sync.dma_start(out=vout[rt][:], in_=vt_sb[:, rt, :])

    nc.compile()
    return nc


def _get_nc():
    if "nc" not in _CACHE:
        _CACHE["nc"] = _build_nc()
    return _CACHE["nc"]


def _prep(x_pred, x_target):
    """Host-side layout: targets sorted by b2 (identity chunk layout)."""
    import ml_dtypes

    b2 = np.einsum(
        "ij,ij->i", x_target.astype(np.float64), x_target.astype(np.float64)
    )
    order = np.argsort(b2, kind="stable")
    a_t = np.ascontiguousarray(2.0 * x_pred.T).astype(ml_dtypes.bfloat16)
    b_t = np.ascontiguousarray(x_target[order].T).astype(ml_dtypes.bfloat16)
    return a_t, b_t, b2, order


def _host_finish(x_pred, x_target, vo, wr, b2, order, k):
    n = x_pred.shape[0]
    a64 = x_pred.astype(np.float64)
    b64 = x_target.astype(np.float64)
    a2 = np.einsum("ij,ij->i", a64, a64)

    # V family: 512 width-16 chunks (consecutive ranks; first = min b2)
    b2min_v = b2[order[np.arange(NVCH) * WIN]].astype(np.float32)
    chv = np.argpartition(-(vo - b2min_v[None, :]), CV, axis=1)[:, :CV]
    tid_v = order[:V_END].reshape(NVCH, WIN)[chv].reshape(n, CV * WIN)

    # W: width-1 near-exact (2ab in fp16); d2 = a2 + b2 - 2ab
    w_ids = order[V_END:]
    d2_w = (
        a2[:, None].astype(np.float32)
        + b2[w_ids].astype(np.float32)[None, :]
        - wr
    )
    chw = np.argpartition(d2_w, KW, axis=1)[:, :KW]
    tid_w = w_ids[chw]

    tids = np.concatenate([tid_v, tid_w], axis=1)
    vals = np.empty((n, k))
    B = 2048
    for s in range(0, n, B):
        t = tids[s : s + B]
        dots = np.einsum("rd,rcd->rc", a64[s : s + B], b64[t], optimize=True)
        d2 = a2[s : s + B, None] + b2[t] - 2.0 * dots
        vals[s : s + B] = np.partition(d2, k - 1, axis=1)[:, :k]
    d = np.sqrt(np.maximum(vals, 0.0))
    return np.float32(np.maximum(d - HINGE, 0.0).mean(dtype=np.float64))


def _host_exact(x_pred, x_target, k):
    """Exact fallback (never expected in practice)."""
    a = x_pred.astype(np.float32)
    b = x_target.astype(np.float32)
    a2 = np.sum(a * a, axis=1)[:, None]
    b2 = np.sum(b * b, axis=1)[None, :]
    out = np.empty((a.shape[0], k), np.float64)
    B = 1024
    for s in range(0, a.shape[0], B):
        d2 = a2[s : s + B] + b2 - 2.0 * (a[s : s + B] @ b.T)
        out[s : s + B] = np.partition(d2, k - 1, axis=1)[:, :k].astype(np.float64)
    d = np.sqrt(np.maximum(out, 0.0))
    return np.float32(np.maximum(d - HINGE, 0.0).mean(dtype=np.float64))


def kernel(x_pred, x_target, top_k=5, _want_results=False):
    from concourse.bass_utils import run_bass_kernel_spmd

    x_pred = np.asarray(x_pred, dtype=np.float32)
    x_target = np.asarray(x_target, dtype=np.float32)
    k = int(top_k)
    if (
        k > KW
        or x_pred.shape != (N_PRED, DIM)
        or x_target.shape != (N_TGT, DIM)
        or not np.isfinite(x_pred).all()
        or not np.isfinite(x_target).all()
        or float(np.abs(x_pred).max()) * float(np.abs(x_target).max()) * DIM
        > 2.0e4
    ):
        return _host_exact(x_pred, x_target, k)

    nc = _get_nc()
    a_t_full, b_t, b2, order = _prep(x_pred, x_target)

    in_maps = []
    for c in range(N_CORES):
        in_maps.append(
            {
                "a_t": np.ascontiguousarray(
                    a_t_full[:, c * ROWS_PER_CORE : (c + 1) * ROWS_PER_CORE]
                ),
                "b_t": b_t,
            }
        )

    res = run_bass_kernel_spmd(nc, in_maps, list(range(N_CORES)))
    vo = np.concatenate(
        [
            res.results[c]["vout"].reshape(ROWS_PER_CORE, NVCH)
            for c in range(N_CORES)
        ],
        axis=0,
    ).astype(np.float32)
    wr = np.concatenate(
        [
            res.results[c]["wraw"].reshape(ROWS_PER_CORE, NW)
            for c in range(N_CORES)
        ],
        axis=0,
    ).astype(np.float32)
    out = _host_finish(x_pred, x_target, vo, wr, b2, order, k)
    if _want_results:
        return out, res
    return out


# revision 8
# speedup vs baseline: 1.5110x; 1.0033x over previous
"""Trainium2 Bass kernel for nn_DensityLoss (retrieval kNN hinge loss).

Computes mean(relu(topk_smallest_dist(x_pred, x_target, k) - 1.0)).

Strategy (8 NeuronCores, SPMD, x_pred rows sharded; 1024 rows/core):
  Every one of the 16384 dot products per row must be drained out of PSUM
  by exactly one compute-engine read (DMA and Pool cannot touch PSUM).
  The drain is split between the DVE and ScalarE, each with its own
  2-deep PSUM tile rotation so both stream independently:

  - V region (positions    0.. 8191): DVE tensor_reduce(axis=X) over a
    [128, 64, 16] PSUM view drains AND folds a 1024-group in one op ->
    512 width-16 chunk maxima per row.
  - W region (positions 8192..16383): ScalarE evacuates fp32->fp16 and
    the slab is shipped raw -> 8192 width-1 near-exact scores per row.

  The kernel runs group-PAIR-major (outer: 8 (V,W) group pairs; inner:
  8 rowtiles), so compute starts as soon as the first b_t slices land
  instead of waiting for the whole 4 MiB load, and the TensorE stays
  continuously busy (p-state).

  Host: targets are b2-sorted (pure sort, no permutation needed: chunk =
  16 consecutive ranks). Chunk score = chunk-max(2ab) - min b2; top-CV
  V-chunks + top-KW W-columns are rescored exactly in float64 -> top-k
  -> hinge -> mean.
"""

import numpy as np

N_CORES = 8
N_PRED = 8192
N_TGT = 16384
DIM = 128
ROWS_PER_CORE = N_PRED // N_CORES  # 1024
ROWTILES = ROWS_PER_CORE // 128    # 8
BANK = 512                         # fp32 PSUM bank, matmul max N
GRP = 1024                         # drain group width (2 PSUM banks)
NPAIR = 8                          # (V, W) group pairs per rowtile
WIN = 16                           # V-chunk width
V_END = 8192                       # V region positions [0, 8192)
NVCH = V_END // WIN                # 512 V-chunks
NW = 8192                          # width-1 raw columns
CV = 12                            # top chunks, V family (width 16)
KW = 16                            # raw width-1 candidates
HINGE = 1.0

_CACHE = {}


def _build_nc():
    import concourse.bacc as bacc
    import concourse.bass as bass
    import concourse.mybir as mybir
    import concourse.tile as tile

    dt = mybir.dt
    nc = bacc.Bacc(
        "TRN2",
        target_bir_lowering=False,
        debug=False,
        num_devices=N_CORES,
    )
    a_t = nc.dram_tensor("a_t", [DIM, ROWS_PER_CORE], dt.bfloat16, kind="ExternalInput")
    b_t = nc.dram_tensor("b_t", [DIM, N_TGT], dt.bfloat16, kind="ExternalInput")
    vout = nc.dram_tensor(
        "vout", [ROWTILES, 128, NVCH], dt.float16, kind="ExternalOutput"
    )
    wraw = nc.dram_tensor(
        "wraw", [ROWTILES, 128, NW], dt.float16, kind="ExternalOutput"
    )

    AX = mybir.AxisListType.X
    MAX = mybir.AluOpType.max

    with tile.TileContext(nc) as tc:
        with (
            tc.tile_pool(name="const", bufs=1) as cpool,
            tc.tile_pool(name="vp", bufs=2, space="PSUM") as vpp,
            tc.tile_pool(name="sp", bufs=2, space="PSUM") as spp,
            tc.tile_pool(name="slab", bufs=4) as spool,
        ):
            bt_sb = cpool.tile([DIM, N_TGT], dt.bfloat16)
            at_sb = cpool.tile([DIM, ROWS_PER_CORE], dt.bfloat16)
            # per-rowtile V-chunk accumulators, DMA'd out at the end
            vt_sb = cpool.tile([128, ROWTILES, NVCH], dt.float16)

            nc.sync.dma_start(out=at_sb[:], in_=a_t[:])
            # b_t in 16 x 1024-col slices, first-use order: pair p uses
            # cols [1024p, 1024p+1024) and [8192+1024p, ...)
            for p in range(NPAIR):
                for half in (0, 1):
                    sl = bass.ts(half * NPAIR + p, GRP)
                    nc.sync.dma_start(out=bt_sb[:, sl], in_=b_t[:, sl])

            for p in range(NPAIR):
                vcols = p * GRP            # V-group column base
                wcols = V_END + p * GRP    # W-group column base
                for rt in range(ROWTILES):
                    lhsT = at_sb[:, bass.ts(rt, 128)]
                    pv = vpp.tile([128, GRP], dt.float32)
                    pw = spp.tile([128, GRP], dt.float32)
                    for j in range(GRP // BANK):
                        nc.tensor.matmul(
                            pv[:, bass.ts(j, BANK)],
                            lhsT,
                            bt_sb[:, bass.ts(vcols // BANK + j, BANK)],
                            start=True,
                            stop=True,
                        )
                    for j in range(GRP // BANK):
                        nc.tensor.matmul(
                            pw[:, bass.ts(j, BANK)],
                            lhsT,
                            bt_sb[:, bass.ts(wcols // BANK + j, BANK)],
                            start=True,
                            stop=True,
                        )
                    nc.vector.tensor_reduce(
                        vt_sb[:, rt, p * (GRP // WIN) : (p + 1) * (GRP // WIN)],
                        pv[:].rearrange("p (c w) -> p c w", w=WIN),
                        axis=AX,
                        op=MAX,
                    )
                    slab = spool.tile([128, GRP], dt.float16)
                    nc.scalar.copy(slab[:], pw[:])
                    # issue output DMAs from the idle Pool sequencer
                    # (SWDGE): ~25 ns issue vs ~605 ns on sync, which
                    # otherwise serializes 64 issues into the critical path
                    nc.gpsimd.dma_start(
                        out=wraw[rt][:, bass.ts(p, GRP)], in_=slab[:]
                    )
            for rt in range(ROWTILES):
                nc.sync.dma_start(out=vout[rt][:], in_=vt_sb[:, rt, :])

    nc.compile()
    return nc


def _get_nc():
    if "nc" not in _CACHE:
        _CACHE["nc"] = _build_nc()
    return _CACHE["nc"]


def _prep(x_pred, x_target):
    """Host-side layout: targets sorted by b2 (identity chunk layout)."""
    import ml_dtypes

    b2 = np.einsum(
        "ij,ij->i", x_target.astype(np.float64), x_target.astype(np.float64)
    )
    order = np.argsort(b2, kind="stable")
    a_t = np.ascontiguousarray(2.0 * x_pred.T).astype(ml_dtypes.bfloat16)
    b_t = np.ascontiguousarray(x_target[order].T).astype(ml_dtypes.bfloat16)
    return a_t, b_t, b2, order


def _host_finish(x_pred, x_target, vo, wr, b2, order, k):
    n = x_pred.shape[0]
    a64 = x_pred.astype(np.float64)
    b64 = x_target.astype(np.float64)
    a2 = np.einsum("ij,ij->i", a64, a64)

    # V family: 512 width-16 chunks (consecutive ranks; first = min b2)
    b2min_v = b2[order[np.arange(NVCH) * WIN]].astype(np.float32)
    chv = np.argpartition(-(vo - b2min_v[None, :]), CV, axis=1)[:, :CV]
    tid_v = order[:V_END].reshape(NVCH, WIN)[chv].reshape(n, CV * WIN)

    # W: width-1 near-exact (2ab in fp16); d2 = a2 + b2 - 2ab
    w_ids = order[V_END:]
    d2_w = (
        a2[:, None].astype(np.float32)
        + b2[w_ids].astype(np.float32)[None, :]
        - wr
    )
    chw = np.argpartition(d2_w, KW, axis=1)[:, :KW]
    tid_w = w_ids[chw]

    tids = np.concatenate([tid_v, tid_w], axis=1)
    vals = np.empty((n, k))
    B = 2048
    for s in range(0, n, B):
        t = tids[s : s + B]
        dots = np.einsum("rd,rcd->rc", a64[s : s + B], b64[t], optimize=True)
        d2 = a2[s : s + B, None] + b2[t] - 2.0 * dots
        vals[s : s + B] = np.partition(d2, k - 1, axis=1)[:, :k]
    d = np.sqrt(np.maximum(vals, 0.0))
    return np.float32(np.maximum(d - HINGE, 0.0).mean(dtype=np.float64))


def _host_exact(x_pred, x_target, k):
    """Exact fallback (never expected in practice)."""
    a = x_pred.astype(np.float32)
    b = x_target.astype(np.float32)
    a2 = np.sum(a * a, axis=1)[:, None]
    b2 = np.sum(b * b, axis=1)[None, :]
    out = np.empty((a.shape[0], k), np.float64)
    B = 1024
    for s in range(0, a.shape[0], B):
        d2 = a2[s : s + B] + b2 - 2.0 * (a[s : s + B] @ b.T)
        out[s : s + B] = np.partition(d2, k - 1, axis=1)[:, :k].astype(np.float64)
    d = np.sqrt(np.maximum(out, 0.0))
    return np.float32(np.maximum(d - HINGE, 0.0).mean(dtype=np.float64))


def kernel(x_pred, x_target, top_k=5, _want_results=False):
    from concourse.bass_utils import run_bass_kernel_spmd

    x_pred = np.asarray(x_pred, dtype=np.float32)
    x_target = np.asarray(x_target, dtype=np.float32)
    k = int(top_k)
    if (
        k > KW
        or x_pred.shape != (N_PRED, DIM)
        or x_target.shape != (N_TGT, DIM)
        or not np.isfinite(x_pred).all()
        or not np.isfinite(x_target).all()
        or float(np.abs(x_pred).max()) * float(np.abs(x_target).max()) * DIM
        > 2.0e4
    ):
        return _host_exact(x_pred, x_target, k)

    nc = _get_nc()
    a_t_full, b_t, b2, order = _prep(x_pred, x_target)

    in_maps = []
    for c in range(N_CORES):
        in_maps.append(
            {
                "a_t": np.ascontiguousarray(
                    a_t_full[:, c * ROWS_PER_CORE : (c + 1) * ROWS_PER_CORE]
                ),
                "b_t": b_t,
            }
        )

    res = run_bass_kernel_spmd(nc, in_maps, list(range(N_CORES)))
    vo = np.concatenate(
        [
            res.results[c]["vout"].reshape(ROWS_PER_CORE, NVCH)
            for c in range(N_CORES)
        ],
        axis=0,
    ).astype(np.float32)
    wr = np.concatenate(
        [
            res.results[c]["wraw"].reshape(ROWS_PER_CORE, NW)
            for c in range(N_CORES)
        ],
        axis=0,
    ).astype(np.float32)
    out = _host_finish(x_pred, x_target, vo, wr, b2, order, k)
    if _want_results:
        return out, res
    return out
